# revision 42
# baseline (speedup 1.0000x reference)
"""Trainium2 Bass kernel for EpisodicMemoryBank (retrieval kNN + soft vote).

Computation (matches the jax reference):
    x_n    = l2norm(x)           # [B, D]   B=1024, D=512
    k_n    = l2norm(keys)        # [M, D]   M=60000
    scores = x_n @ k_n.T         # [B, M]
    top50  = top_k(scores, 50)
    logits[b, c] = sum of top50 scores of class c    # [B, 10]

Distribution: keys/values are sharded across 8 cores along M (7500 each,
zero-padded to 7680 = 60*128).  Each core computes exact fp32 scores for
all 1024 queries against its shard, extracts its local top-56 candidates
per query (hierarchical top-8-per-512-group + drain, with the class label
encoded in the 4 low mantissa bits of each score), exchanges candidates
with an on-device AllToAll so core c receives all candidates for query
block c, then merges (top-50 of 448) and votes.  Host code only shards
inputs and concatenates the 8 per-core [128, 10] outputs.

Precision note: top-50 boundary gaps for this input go down to ~3e-8, so
scores must be exact fp32 (fp32r / bf16 screening flips boundaries and
moves ~0.14 of score mass between classes).  The fp32 matmul is the
compute floor; the pipeline is organized to keep the PE busy and push all
ancillary work to Scalar/GpSimd/DVE in parallel:
  - per-key 1/||k|| is folded into the PE transpose by streaming a
    diagonal matrix diag(1/||k||) instead of the identity
  - transposed d-blocks are assembled 4-at-a-time in one PSUM bank so a
    single [128,512] scalar copy drains them
  - the label encode (score mantissa & ~0xF | label) runs on GpSimd
    reading the matmul PSUM directly; DVE only runs max8 + the
    match_replace drain rounds

Correctness of the hierarchical extraction: a member of the *global*
top-50 misses the per-group top-8 only if >=8 same-group elements exceed
it; those would all be global-top-50 members too, i.e. one 512-column
group would hold >=9 of the 50 (P ~ 1e-4 per full run for random scores).
"""

import sys

for _p in ("/opt/trn_rl_repo", "/root/.axon_site/_ro/trn_rl_repo"):
    if _p not in sys.path:
        sys.path.insert(0, _p)

import numpy as np

import concourse.bass as bass
import concourse.mybir as mybir
from concourse import bass_utils
from concourse.masks import make_identity
from concourse.tile import TileContext

F32 = mybir.dt.float32
U32 = mybir.dt.uint32
U8 = mybir.dt.uint8

B = 1024          # queries
D = 512           # feature dim
M = 60000         # memory size
C = 10            # classes
K = 50            # top-k
NCORES = 8
MS = 7680         # per-core padded shard (60 * 128)
P = 128           # partitions
ND = D // P       # 4 d-blocks
NQ = B // P       # 8 query tiles
CHUNK = 512       # m-chunk per matmul
NCH = MS // CHUNK  # 15 chunks
NGRP = NCH        # level-1 max8 group = one matmul chunk -> G has 120 cols
NSEL = 56         # local candidates extracted (7 rounds x 8)
NROUND = NSEL // 8
NEG_FILL = -1.0e9

MASK_HI = 0xFFFFFFF0  # keep-score mask (clear 4 low mantissa bits)
MASK_LO = 0x0000000F  # label mask


def _split_multi_waits(nc):
    """walrus in this toolchain accepts at most ONE embedded sync wait per
    instruction (setupSyncWait: 'Too many sync wait commands').  Tile attaches
    up to ~13.  Hoist all-but-one wait onto standalone EventSemaphore
    instructions on the same engine queue, immediately before the owner."""
    n = 0
    for bb in nc.main_func.blocks:
        new = []
        for ins in bb.instructions:
            si = ins.sync_info
            if si is not None and si.on_wait and len(si.on_wait) > 1:
                waits = list(si.on_wait)
                for w in waits[:-1]:
                    ev = mybir.InstEventSemaphore(
                        name=f"EVW-{n}",
                        ins=[],
                        outs=[],
                        engine=ins.engine,
                        sync_info=mybir.SyncInfo(on_wait=[w], on_update=[]),
                    )
                    n += 1
                    new.append(ev)
                ins.sync_info = mybir.SyncInfo(
                    on_wait=[waits[-1]], on_update=list(si.on_update)
                )
            new.append(ins)
        bb.instructions[:] = new
    return n


def _build_kernel():
    """Build the SPMD Bass program (same program on all 8 cores)."""
    nc = bass.Bass(
        "TRN2",
        target_bir_lowering=False,
        debug=False,
        num_devices=NCORES,
    )

    x_d = nc.dram_tensor("x", [B, D], F32, kind="ExternalInput")
    keys_d = nc.dram_tensor("keys", [MS, D], F32, kind="ExternalInput")
    keys_t_d = nc.dram_tensor("keys_t", [D, MS], F32, kind="ExternalInput")
    lab_d = nc.dram_tensor("labels_bc", [P, MS], U8, kind="ExternalInput")
    # every core merges+votes ALL 8 query blocks (identical gathered
    # candidates), out col block qt*10..qt*10+10 holds block qt's logits
    out_d = nc.dram_tensor("logits", [P, NQ * C], F32, kind="ExternalOutput")

    with TileContext(nc) as tc:
        with (
            tc.tile_pool(name="big", bufs=1) as big,
            tc.tile_pool(name="io", bufs=3) as io,
            tc.tile_pool(name="scr", bufs=3) as scr,
            tc.tile_pool(name="small", bufs=4) as small,
            tc.tile_pool(name="sel", bufs=2) as sel,
            tc.tile_pool(name="psI", bufs=2, space="PSUM") as psI_pool,
            tc.tile_pool(name="psB", bufs=2, space="PSUM") as psB_pool,
            tc.tile_pool(name="psC", bufs=4, space="PSUM") as psC_pool,
            tc.tile_pool(name="dram", bufs=1, space="DRAM") as dram,
        ):
            a2a_in = dram.tile([B, NSEL], F32, tag="a2a_in")
            ag_out = [
                dram.tile([B, NSEL], F32, tag=f"ag_out{j}", name=f"ag_out{j}")
                for j in range(NQ)
            ]
            ident = big.tile([P, P], F32, tag="ident")
            make_identity(nc, ident)

            # constant columns used as per-partition "scalar" operands
            mask_hi = big.tile([P, 1], U32, tag="mask_hi")
            nc.vector.memset(mask_hi, MASK_HI)
            mask_lo = big.tile([P, 1], U32, tag="mask_lo")
            nc.vector.memset(mask_lo, MASK_LO)
            mask_u8 = big.tile([P, 1], U8, tag="mask_u8")
            nc.vector.memset(mask_u8, 0xF0)
            cls_cols = big.tile([P, C], F32, tag="cls_cols")
            for c in range(C):
                nc.vector.memset(cls_cols[:, c : c + 1], float(c))

            lab_bc = big.tile([P, MS], U8, tag="lab")

            # knT[d][ch]: [128(d-slice), 512] transposed key chunks, DMA'd
            # directly from the host-transposed keys_t; normalized in place
            # below.  Chunk-granular tiles, with each chunk's DMAs issued
            # inside emit_B_chunk so the SP issue queue follows consumption
            # order (61 upfront issues at ~650ns each starved the ACT chain
            # for ~50us otherwise).
            knT = [
                [
                    big.tile([P, CHUNK], F32, tag=f"knT{d}_{ch}", name=f"knT{d}_{ch}")
                    for ch in range(NCH)
                ]
                for d in range(ND)
            ]
            ones1 = big.tile([1, P], F32, tag="ones1")
            nc.vector.memset(ones1, 1.0)
            # xnT_q[qt]: [128(d-slice), 512] normalized transposed queries,
            # col block d*128 holds d-slice d (per-qt tiles: fine-grained deps)
            xnT_q = [
                big.tile([P, D], F32, tag=f"xnT_q{qt}", name=f"xnT_q{qt}")
                for qt in range(NQ)
            ]

            def inv_norm(tile, clamp):
                """rows of tile: [128, 512]; returns [128,1] tile of 1/||row||
                (square+sum on ACT, sqrt on ACT, recip on DVE)."""
                sq = scr.tile([P, D], F32, tag="sq_scr", bufs=1)
                ss = small.tile([P, 1], F32, tag="ss")
                nc.scalar.activation(
                    sq, tile, mybir.ActivationFunctionType.Square, accum_out=ss
                )
                if clamp:
                    # keep zero pad rows finite: 1/sqrt(max(ss,1e-24)) = 1e12
                    nc.vector.tensor_scalar_max(ss, ss, 1e-24)
                nrm = small.tile([P, 1], F32, tag="nrm")
                nc.scalar.sqrt(nrm, ss)
                inv = small.tile([P, 1], F32, tag="inv")
                nc.vector.reciprocal(inv, nrm)
                return inv

            # ---- stage A: queries -> xnT_cat ----
            for qt in range(NQ):
                xt = io.tile([P, D], F32, tag="io512", name="xt")
                nc.sync.dma_start(xt, x_d.ap()[qt * P : (qt + 1) * P, :])
                inv = inv_norm(xt, clamp=False)
                nc.scalar.mul(xt, xt, inv)
                ps = psB_pool.tile([P, D], F32, tag="psb", name="psa")
                for d in range(ND):
                    nc.tensor.transpose(
                        ps[:, d * P : (d + 1) * P], xt[:, d * P : (d + 1) * P], ident
                    )
                nc.scalar.copy(xnT_q[qt], ps)

            # labels are first needed by the encode ~20us in; issue after the
            # latency-critical x loads
            nc.sync.dma_start(lab_bc, lab_d.ap())

            zeros_u = sel.tile([P, K], U32, tag="zeros_u")
            nc.vector.memset(zeros_u, 0)
            logits = sel.tile([P, NQ * C], F32, tag="logits")

            # ---- stage B: per-key 1/||k|| -> inv_ch, then normalize knT ----
            # Norms are computed from the row-major keys exactly as the
            # baseline did (same ACT square+accum / sqrt / recip chain), then
            # transposed to a row via the PE and replicated across partitions
            # with a K=1 outer-product matmul against a ones column.
            def emit_B_chunk(ch):
                for d in range(ND):
                    nc.sync.dma_start(
                        knT[d][ch],
                        keys_t_d.ap()[
                            d * P : (d + 1) * P, ch * CHUNK : (ch + 1) * CHUNK
                        ],
                    )
                psi = psI_pool.tile([1, CHUNK], F32, tag="psi", name="psi")
                for sub in range(CHUNK // P):
                    mt = ch * (CHUNK // P) + sub
                    kt = io.tile([P, D], F32, tag="io512", name="kt")
                    nc.sync.dma_start(kt, keys_d.ap()[mt * P : (mt + 1) * P, :])
                    inv = inv_norm(kt, clamp=True)
                    nc.tensor.transpose(
                        psi[:, sub * P : (sub + 1) * P], inv, ident
                    )
                invT = scr.tile([1, CHUNK], F32, tag="invT", bufs=2, name="invT")
                nc.scalar.copy(invT, psi)
                psb = psB_pool.tile([P, CHUNK], F32, tag="psb", name="psb")
                nc.tensor.matmul(psb, ones1, invT, start=True, stop=True)
                inv_ch = scr.tile([P, CHUNK], F32, tag="inv_ch", bufs=2, name="inv_ch")
                nc.scalar.copy(inv_ch, psb)
                for d in range(ND):
                    # split across GpSimd/DVE so chunk supply (was 5.1us on
                    # GpSimd alone) outpaces the PE's ~4.2us/chunk consumption
                    eng = nc.gpsimd if d < 2 else nc.vector
                    eng.tensor_tensor(
                        out=knT[d][ch],
                        in0=knT[d][ch],
                        in1=inv_ch,
                        op=mybir.AluOpType.mult,
                    )

            # ---- stage C: scores + local selection ----
            def emit_C_chunk(qt, ch, G):
                m0 = ch * CHUNK
                ps = psC_pool.tile([P, CHUNK], F32, tag="mm", name="ps")
                for d in range(ND):
                    # scores[q, m] += xnT[d,:,q].T @ knT[d,:,m]
                    nc.tensor.matmul(
                        ps,
                        xnT_q[qt][:, d * P : (d + 1) * P],
                        knT[d][ch],
                        start=(d == 0),
                        stop=(d == ND - 1),
                    )
                # drain PSUM on ACT, then splice the label into the low nibble
                # of each score's low byte in place: enc_lo = (enc_lo&0xF0)|lab
                enc = scr.tile([P, CHUNK], F32, tag="enc", bufs=3, name="enc")
                nc.scalar.copy(enc, ps)
                enc_lo = enc.bitcast(U8).rearrange(
                    "p (m b) -> p m b", b=4
                )[:, :, 0]
                nc.vector.scalar_tensor_tensor(
                    out=enc_lo,
                    in0=enc_lo,
                    scalar=mask_u8,
                    in1=lab_bc[:, m0 : m0 + CHUNK],
                    op0=mybir.AluOpType.bitwise_and,
                    op1=mybir.AluOpType.bitwise_or,
                )
                nc.vector.max(out=G[:, ch * 8 : ch * 8 + 8], in_=enc)

            def emit_C_post(qt, G):
                # local top-56 drain + ship + all-gather (overlaps compute)
                Xq = sel.tile([P, NSEL], F32, tag="Xq", bufs=2, name="Xq")
                for r in range(NROUND):
                    slot = Xq[:, r * 8 : r * 8 + 8]
                    nc.vector.max(out=slot, in_=G)
                    if r < NROUND - 1:
                        nc.vector.match_replace(
                            out=G, in_to_replace=slot, in_values=G,
                            imm_value=NEG_FILL,
                        )
                nc.sync.dma_start(a2a_in[qt * P : (qt + 1) * P, :], Xq)
                nc.gpsimd.collective_compute(
                    "AllGather",
                    mybir.AluOpType.bypass,
                    replica_groups=[list(range(NCORES))],
                    ins=[a2a_in[qt * P : (qt + 1) * P, :].opt()],
                    outs=[ag_out[qt].opt()],
                )

            # ---- stage D/E: per-block merge (top-50 of 448) + vote ----
            def emit_merge(qt):
                G2 = sel.tile([P, NCORES * NSEL], F32, tag="G2", bufs=2, name="G2")
                nc.sync.dma_start(
                    G2.rearrange("q (j k) -> q j k", k=NSEL),
                    ag_out[qt][:].rearrange("(j q) k -> q j k", q=P),
                )
                M56 = sel.tile([P, NSEL], F32, tag="M56", bufs=2, name="M56")
                for r in range(NROUND):
                    slot = M56[:, r * 8 : r * 8 + 8]
                    nc.vector.max(out=slot, in_=G2)
                    if r < NROUND - 1:
                        nc.vector.match_replace(
                            out=G2, in_to_replace=slot, in_values=G2,
                            imm_value=NEG_FILL,
                        )
                lab_u = sel.tile([P, K], U32, tag="lab_u", bufs=2, name="lab_u")
                nc.vector.scalar_tensor_tensor(
                    out=lab_u,
                    in0=M56[:, :K].bitcast(U32),
                    scalar=mask_lo,
                    in1=zeros_u,
                    op0=mybir.AluOpType.bitwise_and,
                    op1=mybir.AluOpType.bitwise_or,
                )
                val_f = sel.tile([P, K], F32, tag="val_f", bufs=2, name="val_f")
                nc.vector.scalar_tensor_tensor(
                    out=val_f.bitcast(U32),
                    in0=M56[:, :K].bitcast(U32),
                    scalar=mask_hi,
                    in1=zeros_u,
                    op0=mybir.AluOpType.bitwise_and,
                    op1=mybir.AluOpType.bitwise_or,
                )
                lab_f = sel.tile([P, K], F32, tag="lab_f", bufs=2, name="lab_f")
                nc.vector.tensor_copy(lab_f, lab_u)
                vote_scr = sel.tile([P, K], F32, tag="vote_scr", bufs=2, name="vote_scr")
                for c in range(C):
                    # (lab == c) * val, summed over the 50 slots
                    nc.vector.scalar_tensor_tensor(
                        out=vote_scr,
                        in0=lab_f,
                        scalar=cls_cols[:, c : c + 1],
                        in1=val_f,
                        op0=mybir.AluOpType.is_equal,
                        op1=mybir.AluOpType.mult,
                        accum_out=logits[:, qt * C + c : qt * C + c + 1],
                    )

            # Emission order drives the per-engine in-order queues: block qt's
            # merge is emitted two blocks late so its AllGather has landed and
            # the DVE queue never blocks on the collective.
            for ch in range(NCH):
                emit_B_chunk(ch)
            for qt in range(NQ):
                G = sel.tile([P, NGRP * 8], F32, tag="G", bufs=2, name="G")
                for ch in range(NCH):
                    emit_C_chunk(qt, ch, G)
                emit_C_post(qt, G)
                if qt >= 2:
                    emit_merge(qt - 2)
            emit_merge(NQ - 2)
            emit_merge(NQ - 1)
            nc.sync.dma_start(out_d.ap(), logits)

    return nc


_NC_CACHE = None


def _get_nc():
    global _NC_CACHE
    if _NC_CACHE is None:
        _NC_CACHE = _build_kernel()
    return _NC_CACHE


def _prep_in_maps(x, keys, values):
    x = np.ascontiguousarray(np.asarray(x, dtype=np.float32))
    keys = np.asarray(keys, dtype=np.float32)
    values = np.asarray(values).astype(np.int64)

    mpc = M // NCORES  # 7500 real keys per core
    in_maps = []
    for c in range(NCORES):
        kshard = np.zeros((MS, D), dtype=np.float32)
        kshard[:mpc] = keys[c * mpc : (c + 1) * mpc]
        lab = np.zeros((MS,), dtype=np.uint8)
        lab[:mpc] = values[c * mpc : (c + 1) * mpc].astype(np.uint8)
        lab_bc = np.ascontiguousarray(np.broadcast_to(lab[None, :], (P, MS)))
        in_maps.append(
            {
                "x": x,
                "keys": kshard,
                "keys_t": np.ascontiguousarray(kshard.T),
                "labels_bc": lab_bc,
            }
        )
    return in_maps


LAST_RESULTS = None


def kernel(x, keys, values, k, num_classes):
    assert int(k) == K and int(num_classes) == C
    x = np.asarray(x)
    assert x.shape == (B, D) and np.asarray(keys).shape == (M, D)

    nc = _get_nc()
    if not getattr(nc, "_waits_split", False):
        _split_multi_waits(nc)
        nc._waits_split = True
    in_maps = _prep_in_maps(x, keys, values)
    import os
    res = bass_utils.run_bass_kernel_spmd(
        nc,
        in_maps,
        core_ids=list(range(NCORES)),
        trace=bool(os.environ.get("KERNEL_TRACE")),
    )
    global LAST_RESULTS
    LAST_RESULTS = res
    # core 0 computed all 8 blocks: [128, 8*10] -> [1024, 10]
    lg = np.asarray(res.results[0]["logits"]).reshape(P, NQ, C)
    out = np.ascontiguousarray(lg.transpose(1, 0, 2).reshape(B, C))
    return out.astype(np.float32)
